# revision 28
# baseline (speedup 1.0000x reference)
"""LEFTNet message-passing layer on 8 Trainium2 NeuronCores.

Strategy (graph/data parallel per the sharding hint):
  - Sort edges by target node; core c owns targets [c*2500, (c+1)*2500) and all
    edges pointing at them, so the segment-sum stays local to each core.
  - Each core computes the node-MLP (LayerNorm -> silu -> proj) for its own
    2500-node shard, then an AllGather replicates the full xh table (bf16).
  - Per-edge pipeline: dir-MLP + rbf projection as bf16 matmuls with features
    on partitions, per-edge gathers of xh via dma_gather, elementwise message
    assembly, and the segment sum done as one-hot matmuls accumulating in PSUM
    (edges on the contraction axis), one 128-target block at a time.
  - Host-side work is sharding only: permute/pad/transpose/cast the edge
    tensors, fold constant scales into weights, build gather indices.
"""

import math
import os
import sys

import numpy as np

sys.path.insert(0, "/opt/trn_rl_repo")

N, H, R = 20000, 128, 32
E = 320000
NCORES = 8
NT = N // NCORES          # targets per core
TBLK = 128                # target block (PSUM partition dim)
NBLK = (NT + TBLK - 1) // TBLK   # 20 blocks (last has 68 targets)
SUB = 128                 # edges per subtile (matmul K)
GRP = 4                   # subtiles per group (512 edges)
INV_SQRT_3 = 1.0 / math.sqrt(3.0)
INV_SQRT_H = 1.0 / math.sqrt(H)

_CACHE = {}


def _preprocess(x, vec, edge_index, edge_rbf, weight, edge_vector, edge_cross,
                ln_w, ln_b, xp_w1, xp_w2, rbf_w, dir_w1, dir_b1, dir_w2, dir_b2):
    import ml_dtypes
    bf16 = ml_dtypes.bfloat16

    jj = np.asarray(edge_index[0]).astype(np.int64)
    ii = np.asarray(edge_index[1]).astype(np.int64)
    order = np.argsort(ii, kind="stable")
    ii_s = ii[order]
    jj_s = jj[order]

    # scale folding (host, weights only)
    rbf_w2 = np.asarray(rbf_w, np.float32).copy()
    rbf_w2[:, H:2 * H] *= np.float32(INV_SQRT_3 * INV_SQRT_H)
    w1p = (np.asarray(ln_w, np.float32)[:, None] * np.asarray(xp_w1, np.float32))
    bias1_row = np.asarray(ln_b, np.float32) @ np.asarray(xp_w1, np.float32)  # [H]

    ev = np.asarray(edge_vector, np.float32) * np.float32(INV_SQRT_H)
    ec = np.asarray(edge_cross, np.float32) * np.float32(INV_SQRT_H)

    # per-core segments (edges sorted by target)
    bounds = np.searchsorted(ii_s, np.arange(NCORES + 1) * NT)

    # per-(core, block) counts -> uniform K subtiles per block across all cores
    sub_needed = 0
    blk_counts = np.zeros((NCORES, NBLK), np.int64)
    for c in range(NCORES):
        seg = slice(bounds[c], bounds[c + 1])
        loc = ii_s[seg] - c * NT
        cnt = np.bincount(loc // TBLK, minlength=NBLK)
        blk_counts[c] = cnt
    K = int(max(1, np.max((blk_counts + SUB - 1) // SUB)))
    SUBS = NBLK * K
    EPAD = SUBS * SUB
    NGRP = (SUBS + GRP - 1) // GRP
    assert SUBS % GRP == 0

    weight_f = np.asarray(weight, np.float32)
    rbf_f = np.asarray(edge_rbf, np.float32)
    vec_f = np.asarray(vec, np.float32)

    per_core = []
    for c in range(NCORES):
        seg = slice(bounds[c], bounds[c + 1])
        e_ids = order[seg]              # original edge ids, sorted by target
        loc = (ii_s[seg] - c * NT).astype(np.int64)
        blk = loc // TBLK

        # padded slot for each real edge: block b occupies subtiles [b*K,(b+1)*K)
        pos_in_blk = np.arange(len(loc)) - np.searchsorted(blk, blk)  # rank in block
        slot = blk * (K * SUB) + pos_in_blk
        real = np.zeros(EPAD, np.bool_)
        real[slot] = True

        def scatter(src_rows, fill=0.0, dtype=np.float32, width=None):
            w_ = src_rows.shape[1] if width is None else width
            out = np.full((EPAD, w_), fill, dtype)
            out[slot] = src_rows
            return out

        w_pad = np.zeros((EPAD, 512), np.float32)
        w_pad[slot, :416] = weight_f[e_ids]
        # weightT tiles: [NGRP, 128(wfeat%128), 4(k)*512(edge)]
        wT = np.ascontiguousarray(
            w_pad.reshape(NGRP, GRP * SUB, 4, 128).transpose(0, 3, 2, 1)
        ).astype(bf16).reshape(NGRP, 128, 4 * 512)

        r_pad = scatter(rbf_f[e_ids])
        rbfT = np.ascontiguousarray(
            r_pad.reshape(NGRP, GRP * SUB, R).transpose(0, 2, 1)
        ).astype(bf16)

        # vjj tiles: [NGRP, 128(edge%128), 4(s)*384(f)]
        vjj = np.ascontiguousarray(
            scatter(vec_f[jj_s[seg]].reshape(-1, 3 * H)).astype(bf16)
            .reshape(NGRP, GRP, SUB, 3 * H).transpose(0, 2, 1, 3)
        ).reshape(NGRP, 128, 4 * 384)

        meta = np.zeros((EPAD, 8), np.float32)
        meta[slot, 0:3] = ev[e_ids]
        meta[slot, 3:6] = ec[e_ids]
        tloc = np.full(EPAD, -1.0, np.float32)
        tloc[slot] = (loc % TBLK).astype(np.float32)
        meta[:, 6] = tloc

        meta = np.ascontiguousarray(
            meta.reshape(NGRP, GRP, SUB, 8).transpose(0, 2, 1, 3)
        ).reshape(NGRP, 128, 4 * 8)

        jj_pad = np.zeros(EPAD, np.int64)
        jj_pad[slot] = jj_s[seg]
        ii_pad = np.zeros(EPAD, np.int64)
        ii_pad[slot] = ii_s[seg]
        idx = np.zeros((NGRP, 128, 8), np.int32)
        jr = jj_pad.reshape(NGRP, GRP, SUB)
        ir = ii_pad.reshape(NGRP, GRP, SUB)
        for sl in range(GRP):
            idx[:, :, sl * 2] = jr[:, sl, :].astype(np.int32)
            idx[:, :, sl * 2 + 1] = ir[:, sl, :].astype(np.int32)

        ntile = (NT + 127) // 128
        xs_rows = np.zeros((ntile * 128, H), np.float32)
        xs_rows[:NT] = np.asarray(x, np.float32)[c * NT:(c + 1) * NT]
        xs = np.ascontiguousarray(
            xs_rows.reshape(ntile, 128, H).transpose(1, 0, 2)).reshape(128, ntile * H)

        per_core.append(dict(
            wT=wT, rbfT=rbfT, vjj=vjj, meta=meta, idx=idx, x=xs,
        ))

    consts = dict(
        w1p=w1p.astype(bf16),                                    # [128,128]
        bias1=bias1_row.astype(bf16).reshape(1, H),              # [1,128]
        w2=np.asarray(xp_w2, np.float32).astype(bf16),           # [128,384]
        dw1=np.ascontiguousarray(
            np.vstack([np.asarray(dir_w1, np.float32),
                       np.zeros((512 - 416, 384), np.float32)])
            .reshape(4, 128, 384).transpose(1, 0, 2)).astype(bf16)
            .reshape(128, 4 * 384),                              # [128,4*384]
        dw2=np.ascontiguousarray(
            np.asarray(dir_w2, np.float32)
            .reshape(3, 128, 384).transpose(1, 0, 2)).astype(bf16)
            .reshape(128, 3 * 384),                              # [128,3*384]
        rbfw=rbf_w2.astype(bf16),                                # [32,384]
        b1=np.asarray(dir_b1, np.float32).reshape(3, 128).T.copy(),   # [128,3]
        b2row=np.asarray(dir_b2, np.float32).astype(bf16).reshape(1, 384),
    )
    return per_core, consts, dict(K=K, SUBS=SUBS, EPAD=EPAD, NGRP=NGRP)


def _build(meta_cfg, consts):
    from contextlib import ExitStack
    import concourse.bass as bass
    import concourse.bacc as bacc
    import concourse.mybir as mybir
    import concourse.tile as tile
    from concourse import library_config
    from concourse.masks import make_identity

    dt = mybir.dt
    AT = mybir.AluOpType
    AF = mybir.ActivationFunctionType
    AX = mybir.AxisListType

    K = meta_cfg["K"]; SUBS = meta_cfg["SUBS"]
    EPAD = meta_cfg["EPAD"]; NGRP = meta_cfg["NGRP"]

    nc = bacc.Bacc("TRN2", num_devices=NCORES)

    # -------- DRAM I/O --------
    wT_d = nc.dram_tensor("wT", (NGRP, 128, 4 * 512), dt.bfloat16, kind="ExternalInput")
    rbfT_d = nc.dram_tensor("rbfT", (NGRP, R, 512), dt.bfloat16, kind="ExternalInput")
    vjj_d = nc.dram_tensor("vjj", (NGRP, 128, 4 * 384), dt.bfloat16, kind="ExternalInput")
    meta_d = nc.dram_tensor("meta", (NGRP, 128, 4 * 8), dt.float32, kind="ExternalInput")
    idx_d = nc.dram_tensor("idx", (NGRP, 128, 8), dt.int32, kind="ExternalInput")
    x_d = nc.dram_tensor("x", (128, ((NT + 127) // 128) * H), dt.float32, kind="ExternalInput")
    w1p_d = nc.dram_tensor("w1p", (H, H), dt.bfloat16, kind="ExternalInput")
    bias1_d = nc.dram_tensor("bias1", (1, H), dt.bfloat16, kind="ExternalInput")
    w2_d = nc.dram_tensor("w2", (H, 384), dt.bfloat16, kind="ExternalInput")
    dw1_d = nc.dram_tensor("dw1", (128, 4 * 384), dt.bfloat16, kind="ExternalInput")
    dw2_d = nc.dram_tensor("dw2", (128, 3 * 384), dt.bfloat16, kind="ExternalInput")
    rbfw_d = nc.dram_tensor("rbfw", (R, 384), dt.bfloat16, kind="ExternalInput")
    b1_d = nc.dram_tensor("b1", (H, 3), dt.float32, kind="ExternalInput")
    b2row_d = nc.dram_tensor("b2row", (1, 384), dt.bfloat16, kind="ExternalInput")
    out_d = nc.dram_tensor("out", (NT, 512), dt.float32, kind="ExternalOutput")

    cc_in = nc.dram_tensor("cc_in", (NT, 384), dt.bfloat16)
    cc_out = nc.dram_tensor("cc_out", (N, 384), dt.bfloat16, addr_space="Shared")

    with tile.TileContext(nc) as tc, ExitStack() as ctx:
        const_p = ctx.enter_context(tc.tile_pool(name="const", bufs=1))
        sb = ctx.enter_context(tc.tile_pool(name="sb", bufs=3))
        sb2 = ctx.enter_context(tc.tile_pool(name="sb2", bufs=2))
        msg_p = ctx.enter_context(tc.tile_pool(name="msg", bufs=3))
        p1 = ctx.enter_context(tc.tile_pool(name="p1", bufs=3, space="PSUM"))
        p2 = ctx.enter_context(tc.tile_pool(name="p2", bufs=2, space="PSUM"))
        pr = ctx.enter_context(tc.tile_pool(name="pr", bufs=1, space="PSUM"))
        ps = ctx.enter_context(tc.tile_pool(name="ps", bufs=2, space="PSUM"))

        # ---- constants into SBUF ----
        ident = const_p.tile([128, 128], dt.bfloat16)
        make_identity(nc, ident[:])
        iota = const_p.tile([128, 128], dt.float32)
        nc.gpsimd.iota(iota[:], pattern=[[1, 128]], base=0, channel_multiplier=0,
                       allow_small_or_imprecise_dtypes=True)
        ones1 = const_p.tile([1, 128], dt.bfloat16)
        nc.gpsimd.memset(ones1[:], 1.0)
        eps_c = const_p.tile([128, 1], dt.float32)
        nc.gpsimd.memset(eps_c[:], 1e-5)

        w1p_s = const_p.tile([H, H], dt.bfloat16)
        nc.sync.dma_start(w1p_s[:], w1p_d[:, :])
        bias1_s = const_p.tile([1, H], dt.bfloat16)
        nc.sync.dma_start(bias1_s[:], bias1_d[:, :])
        w2_s = const_p.tile([H, 384], dt.bfloat16)
        nc.sync.dma_start(w2_s[:], w2_d[:, :])
        dw1_s = const_p.tile([128, 4 * 384], dt.bfloat16)
        nc.sync.dma_start(dw1_s[:], dw1_d[:, :])
        dw2_s = const_p.tile([128, 3 * 384], dt.bfloat16)
        nc.sync.dma_start(dw2_s[:], dw2_d[:, :])
        rbfw_s = const_p.tile([R, 384], dt.bfloat16)
        nc.sync.dma_start(rbfw_s[:], rbfw_d[:, :])
        b1_s = const_p.tile([H, 3], dt.float32)
        nc.sync.dma_start(b1_s[:], b1_d[:, :])
        b2row_s = const_p.tile([1, 384], dt.bfloat16)
        nc.sync.dma_start(b2row_s[:], b2row_d[:, :])


        # ---- phase 1: node MLP on this core's shard -> cc_in (bf16) ----
        ntile = (NT + 127) // 128
        x_all = const_p.tile([128, ntile * H], dt.float32)
        nc.sync.dma_start(x_all[:], x_d[:, :])
        for t in range(ntile):
            r0 = t * 128
            nr = min(128, NT - r0)
            xt = x_all[:, t * H:(t + 1) * H]
            mu = sb2.tile([128, 1], dt.float32, tag="mu")
            nc.vector.tensor_reduce(mu[:nr], xt[:nr, :], axis=AX.X, op=AT.add)
            nmu = sb2.tile([128, 1], dt.float32, tag="nmu")
            nc.vector.tensor_scalar_mul(nmu[:nr], mu[:nr], -1.0 / H)
            xc = sb2.tile([128, H], dt.float32, tag="xc")
            nc.vector.tensor_scalar_add(xc[:nr, :], xt[:nr, :], nmu[:nr])
            junk = sb2.tile([128, H], dt.float32, tag="junk")
            ssq = sb2.tile([128, 1], dt.float32, tag="ssq")
            nc.vector.scalar_tensor_tensor(
                out=junk[:nr, :], in0=xc[:nr, :], scalar=1.0, in1=xc[:nr, :],
                op0=AT.mult, op1=AT.mult, accum_out=ssq[:nr])
            std = sb2.tile([128, 1], dt.float32, tag="std")
            nc.scalar.activation(std[:nr], ssq[:nr], AF.Sqrt,
                                 bias=eps_c[:nr], scale=1.0 / H)
            rstd = sb2.tile([128, 1], dt.float32, tag="rstd")
            nc.vector.reciprocal(rstd[:nr], std[:nr])
            xn = sb2.tile([128, H], dt.bfloat16, tag="xn")
            nc.scalar.activation(xn[:nr, :], xc[:nr, :], AF.Copy,
                                 bias=0.0, scale=rstd[:nr])
            # transpose -> [H, nr]
            ptr = p1.tile([128, 128], dt.bfloat16, tag="p1")
            nc.tensor.transpose(ptr[:, :nr], xn[:nr, :], ident[:nr, :nr])
            xnT = sb2.tile([128, 128], dt.bfloat16, tag="xnT")
            nc.vector.tensor_copy(xnT[:, :nr], ptr[:, :nr])
            ph1 = pr.tile([128, 128], dt.float32, tag="pr")
            nc.tensor.matmul(ph1[:nr, :], xnT[:, :nr], w1p_s[:, :],
                             start=True, stop=False)
            nc.tensor.matmul(ph1[:nr, :], ones1[:, :nr], bias1_s[:, :],
                             start=False, stop=True)
            h1 = sb2.tile([128, H], dt.bfloat16, tag="h1")
            nc.scalar.activation(h1[:nr, :], ph1[:nr, :], AF.Silu)
            pt2 = p1.tile([128, 128], dt.bfloat16, tag="p1")
            nc.tensor.transpose(pt2[:, :nr], h1[:nr, :], ident[:nr, :nr])
            h1T = sb2.tile([128, 128], dt.bfloat16, tag="h1T")
            nc.vector.tensor_copy(h1T[:, :nr], pt2[:, :nr])
            ph2 = p2.tile([128, 384], dt.float32, tag="p2")
            nc.tensor.matmul(ph2[:nr, :], h1T[:, :nr], w2_s[:, :],
                             start=True, stop=True)
            xh = sb2.tile([128, 384], dt.bfloat16, tag="xh")
            nc.vector.tensor_copy(xh[:nr, :], ph2[:nr, :])
            nc.sync.dma_start(cc_in[r0:r0 + nr, :], xh[:nr, :])

        # ---- all-gather xh ----
        nc.gpsimd.collective_compute(
            "AllGather", AT.bypass,
            replica_groups=[list(range(NCORES))],
            ins=[cc_in[:, :]], outs=[cc_out[:, :]],
        )

        # ---- phase 2: edge pipeline ----
        for g in range(NGRP):
            wt = sb.tile([128, 4 * 512], dt.bfloat16, tag="wt")
            nc.sync.dma_start(wt[:], wT_d[g, :, :])
            rbfT = sb.tile([R, 512], dt.bfloat16, tag="rbfT")
            nc.sync.dma_start(rbfT[:], rbfT_d[g, :, :])
            vjj = sb.tile([128, 4 * 384], dt.bfloat16, tag="vjj")
            nc.sync.dma_start(vjj[:], vjj_d[g, :, :])
            met = sb.tile([128, 4 * 8], dt.float32, tag="met")
            nc.sync.dma_start(met[:], meta_d[g, :, :])
            idxt = sb.tile([128, 8], dt.int32, tag="idxt")
            nc.sync.dma_start(idxt[:], idx_d[g, :, :])
            xj = sb.tile([128, 4 * 384], dt.bfloat16, tag="xj")
            xi = sb.tile([128, 4 * 384], dt.bfloat16, tag="xi")
            for sl in range(4):
                nc.gpsimd.indirect_dma_start(
                    out=xj[:, sl * 384:(sl + 1) * 384], out_offset=None,
                    in_=cc_out[:, :],
                    in_offset=bass.IndirectOffsetOnAxis(
                        ap=idxt[:, sl * 2:sl * 2 + 1], axis=0))
                nc.gpsimd.indirect_dma_start(
                    out=xi[:, sl * 384:(sl + 1) * 384], out_offset=None,
                    in_=cc_out[:, :],
                    in_offset=bass.IndirectOffsetOnAxis(
                        ap=idxt[:, sl * 2 + 1:sl * 2 + 2], axis=0))

            xs_sum = msg_p.tile([128, 4 * 384], dt.bfloat16, tag="xs")
            nc.vector.tensor_tensor(out=xs_sum[:], in0=xj[:], in1=xi[:], op=AT.add)

            # MLP1 (features on partitions): h1T[g] = silu(dw1.T @ wT + b1)
            h1t = msg_p.tile([128, 3 * 512], dt.bfloat16, tag="h1t")
            for m in range(3):
                pm = p1.tile([128, 512], dt.float32, tag="p1")
                for k in range(4):
                    nc.tensor.matmul(
                        pm[:, :], dw1_s[:, k * 384 + m * 128:k * 384 + m * 128 + 128],
                        wt[:, k * 512:(k + 1) * 512],
                        start=(k == 0), stop=(k == 3))
                nc.scalar.activation(h1t[:, m * 512:(m + 1) * 512], pm[:, :],
                                     AF.Silu, bias=b1_s[:, m:m + 1])

            oh = msg_p.tile([128, 4 * 128], dt.bfloat16, tag="oh")
            for s in range(4):
                nc.vector.tensor_tensor(
                    out=oh[:, s * 128:(s + 1) * 128],
                    in0=met[:, s * 8 + 6:s * 8 + 7].to_broadcast([128, 128]),
                    in1=iota[:], op=AT.is_equal)

            for s in range(4):
                sub = g * 4 + s          # global subtile id
                b = sub // K             # target block
                sb_in_b = sub % K
                nrow = min(TBLK, NT - b * TBLK)

                ph2e = p2.tile([128, 384], dt.float32, tag="p2")
                for k in range(3):
                    nc.tensor.matmul(
                        ph2e[:, :], h1t[:, k * 512 + s * 128:k * 512 + s * 128 + 128],
                        dw2_s[:, k * 384:(k + 1) * 384],
                        start=(k == 0), stop=False)
                nc.tensor.matmul(ph2e[:, :], ones1[:, :], b2row_s[:, :],
                                 start=False, stop=True)
                prb = pr.tile([128, 384], dt.float32, tag="pr")
                nc.tensor.matmul(prb[:, :], rbfT[:, s * 128:(s + 1) * 128],
                                 rbfw_s[:, :], start=True, stop=True)
                h2 = msg_p.tile([128, 384], dt.bfloat16, tag="h2")
                nc.scalar.activation(h2[:, :], ph2e[:, :], AF.Copy)
                rbfh = msg_p.tile([128, 384], dt.bfloat16, tag="rbfh")
                nc.vector.tensor_tensor(out=rbfh[:], in0=prb[:, :], in1=h2[:],
                                        op=AT.mult)
                m_t = msg_p.tile([128, 384], dt.bfloat16, tag="m_t")
                nc.vector.tensor_tensor(
                    out=m_t[:], in0=xs_sum[:, s * 384:(s + 1) * 384],
                    in1=rbfh[:], op=AT.mult)

                # vector messages
                va = msg_p.tile([128, 384], dt.bfloat16, tag="va")
                vb = msg_p.tile([128, 384], dt.bfloat16, tag="vb")
                vm = msg_p.tile([128, 384], dt.bfloat16, tag="vm")
                for c in range(3):
                    cs = slice(c * 128, (c + 1) * 128)
                    nc.vector.tensor_tensor(
                        out=va[:, cs],
                        in0=vjj[:, s * 384 + c * 128:s * 384 + c * 128 + 128],
                        in1=m_t[:, 128:256], op=AT.mult)
                    nc.vector.scalar_tensor_tensor(
                        out=vb[:, cs], in0=m_t[:, 256:384],
                        scalar=met[:, s * 8 + c:s * 8 + c + 1],
                        in1=va[:, cs], op0=AT.mult, op1=AT.add)
                    nc.vector.scalar_tensor_tensor(
                        out=vm[:, cs], in0=m_t[:, 0:128],
                        scalar=met[:, s * 8 + 3 + c:s * 8 + 4 + c],
                        in1=vb[:, cs], op0=AT.mult, op1=AT.add)

                # scatter-accumulate into the block PSUM
                if sb_in_b == 0:
                    blk_ps = ps.tile([128, 512], dt.float32, tag="ps")
                    _live_ps = blk_ps
                else:
                    blk_ps = _live_ps
                first = sb_in_b == 0
                last = sb_in_b == K - 1
                ohs = oh[:, s * 128:(s + 1) * 128]
                nc.tensor.matmul(blk_ps[:, 0:128], ohs, m_t[:, 0:128],
                                 start=first, stop=False, skip_group_check=True)
                nc.tensor.matmul(blk_ps[:, 128:512], ohs, vm[:, :],
                                 start=False, stop=last, skip_group_check=True)
                if last:
                    ot = msg_p.tile([128, 512], dt.float32, tag="ot")
                    nc.scalar.activation(ot[:nrow, :], blk_ps[:nrow, :], AF.Copy)
                    nc.sync.dma_start(out_d[b * TBLK:b * TBLK + nrow, :],
                                      ot[:nrow, :])

    nc.compile()
    return nc


def kernel(x, vec, edge_index, edge_rbf, weight, edge_vector, edge_cross,
           ln_w, ln_b, xp_w1, xp_w2, rbf_w, dir_w1, dir_b1, dir_w2, dir_b2,
           _trace=False):
    from concourse.bass_utils import run_bass_kernel_spmd

    per_core, consts, cfg = _preprocess(
        x, vec, edge_index, edge_rbf, weight, edge_vector, edge_cross,
        ln_w, ln_b, xp_w1, xp_w2, rbf_w, dir_w1, dir_b1, dir_w2, dir_b2)

    key = (cfg["K"],)
    if key not in _CACHE:
        _CACHE[key] = _build(cfg, consts)
    nc = _CACHE[key]

    in_maps = []
    for c in range(NCORES):
        d = dict(per_core[c])
        d.update(consts)
        in_maps.append(d)

    try:
        res = run_bass_kernel_spmd(nc, in_maps, core_ids=list(range(NCORES)),
                                   trace=_trace)
    except ModuleNotFoundError:
        res = run_bass_kernel_spmd(nc, in_maps, core_ids=list(range(NCORES)),
                                   trace=False)
    dx = np.concatenate([res.results[c]["out"][:, :H] for c in range(NCORES)], 0)
    dvec = np.concatenate(
        [res.results[c]["out"][:, H:].reshape(NT, 3, H) for c in range(NCORES)], 0)
    if _trace:
        kernel._last_exec_ns = res.exec_time_ns
        kernel._last_results = res
    return dx.astype(np.float32), dvec.astype(np.float32)
